# revision 6
# baseline (speedup 1.0000x reference)
"""Marching-cubes mesh decoder on 8 Trainium2 NeuronCores.

Sharding: cube-x slabs (16 cube layers per core).  Per core:
  - bit-pack the 8 corner sign bits into a case id per cube with 8 PE
    shift-matmuls accumulated in PSUM (c in [0,256), exact in fp32),
  - one gpsimd ap_gather per cube layer looks up a per-case table
    (G_s gamma values for the 15 triangle slots + 5 mask bits),
  - faces = 16384x + 128y + z + G - 128x*h1 - y*h2 assembled in 5 DVE
    passes, where h1 = (G >= y_off), h2 = (G >= z_off) are recovered from
    the gathered value itself,
  - verts: edge interpolation t = s1/(s1-s2) with the zero-denominator
    guard, written as interleaved [x|y|z] rows.
All table/constant inputs are host-precomputed from cases/masks/min_point/
size only; `field` is never inspected on the host.
"""

import sys
import types
import contextlib
import functools

import numpy as np

sys.path.insert(0, "/opt/trn_rl_repo")

GRID = 128
NC = 8            # NeuronCores
XL = 16           # cube layers per core
NCOL = 16 * 127   # gather stream columns per group per layer (y_in fast)
Y_OFF = (GRID - 1) * GRID * GRID          # 2080768
Z_OFF = 2 * Y_OFF                          # 4161536


def _ensure_ntff_hook():
    """Register the NTFF profile hook if the image's antenv lacks it."""
    try:
        import antenv.axon_hooks  # noqa: F401
        return
    except ImportError:
        pass
    mod = types.ModuleType("antenv.axon_hooks")
    mod._hook = None
    mod.set_axon_ntff_profile_hook = lambda h: setattr(mod, "_hook", h)
    mod.get_axon_ntff_profile_hook = lambda: mod._hook
    sys.modules["antenv.axon_hooks"] = mod
    import antenv

    antenv.axon_hooks = mod
    with contextlib.suppress(Exception):
        from trn_agent_boot.trn_boot import _ntff_profile_via_ctypes

        hook = _ntff_profile_via_ctypes("/opt/axon/libaxon_pjrt.so")
        if hook is not None:
            mod._hook = hook


_ensure_ntff_hook()

import concourse.bass as bass                     # noqa: E402
import concourse.tile as tile                     # noqa: E402
from concourse import bacc, mybir                 # noqa: E402
from concourse.bass_utils import run_bass_kernel_spmd  # noqa: E402

F32 = mybir.dt.float32
BF16 = mybir.dt.bfloat16
I16 = mybir.dt.int16
I32 = mybir.dt.int32
U8 = mybir.dt.uint8
AL = mybir.AluOpType
ACTF = mybir.ActivationFunctionType


# ---------------------------------------------------------------- tables

def build_tables(cases, masks):
    """Per-partition gather tables [128, 512] f32.

    partition p = 16*g + s (content identical for all 8 groups g):
      s in [0,15): tab[p, 2c]   = G_s(c)   (gamma for slot s)
                   tab[p, 2c+1] = int-bits of masks[c, s] for s < 5 else 0
    """
    cases = np.asarray(cases).reshape(256, 15).astype(np.int64)
    masks = np.asarray(masks).reshape(256, 5).astype(np.int32)
    inner = np.array([
        [0, 128, 1, 129],
        [0, 16256, 1, 16257],
        [0, 16256, 127, 16383],
    ], np.int64)
    base = np.array([0, Y_OFF, Z_OFF], np.int64)
    tab = np.zeros((128, 512), np.float32)
    for s in range(15):
        e = cases[:, s]
        h = e >> 2
        g = base[h] + inner[h, e & 3]
        tab[s, 0::2] = g.astype(np.float32)
        if s < 5:
            tab[s, 1::2] = masks[:, s].astype(np.float32)
    for g_ in range(1, 8):
        tab[16 * g_:16 * g_ + 16] = tab[0:16]
    return tab


def build_wmats():
    """8 scaled shift matrices [8, 128, 128] for the case bit-pack.

    c[y, z] = sum over (dx,dy,dz) of 2^(dx + 2 dy + 4 dz) * pos[x+dx, y+dy, z+dz]
    matmul m: lhsT[j, y] = coef * (j == y + dy), rhs = pos plane x+dx,
    column offset dz.
    """
    import ml_dtypes
    w = np.zeros((8, 128, 128), np.float32)
    m = 0
    for dz in (0, 1):
        for dy in (0, 1):
            for dx in (0, 1):
                coef = float(1 << (dx + 2 * dy + 4 * dz))
                jj = np.arange(dy, 128)
                w[m, jj, jj - dy] = coef
                m += 1
    return w.astype(ml_dtypes.bfloat16)


def build_consts(min_point, size, x0):
    """Host-built constant tiles for the core with cube-x base x0."""
    mp = np.asarray(min_point, np.float64)
    sz = np.asarray(size, np.float64)
    d0, d1, d2 = (sz / (GRID - 1)).astype(np.float64)
    m0, m1, m2 = mp

    p = np.arange(128)
    i = np.arange(NCOL)
    yv = 16 * (p[:, None] // 16) + (i[None, :] % 16)       # y(p, i)
    zv = np.broadcast_to(i[None, :] // 16, (128, NCOL))     # z(i)
    bz = (128.0 * yv + zv).astype(np.float32)
    yt = yv.astype(np.float32)

    xs = x0 + np.arange(XL)
    x128 = np.broadcast_to((-128.0 * xs)[None, :], (128, XL)).astype(np.float32).copy()
    x16k = np.broadcast_to((16384.0 * xs)[None, :], (128, XL)).astype(np.float32).copy()
    xs0 = np.broadcast_to((xs * d0 + m0)[None, :], (128, XL)).astype(np.float32).copy()

    k = np.arange(128)
    c0x = np.zeros((128, 384), np.float32)
    c0x[:, 1::3] = (p * d1 + m1)[:, None]
    c0x[:, 2::3] = (k * d2 + m2)[None, :]
    c0y = np.zeros((128, 384), np.float32)
    c0y[:, 1::3] = (p * d1 + m1)[:, None]
    c0y[:, 2::3] = (k * d2 + m2)[None, :]
    c0z = np.zeros((128, 381), np.float32)
    c0z[:, 1::3] = (p * d1 + m1)[:, None]
    c0z[:, 2::3] = (k[:127] * d2 + m2)[None, :]
    return dict(bz=bz, yt=yt, x128=x128, x16k=x16k, xs0=xs0,
                c0x=c0x, c0y=c0y, c0z=c0z,
                deltas=(float(d0), float(d1), float(d2)))


# ---------------------------------------------------------------- program

def _c3(ap, c):
    """Select coordinate slot c of an interleaved [p, 3k+c] view."""
    return ap.rearrange("p (k c) -> p k c", c=3)[:, :, c]


def build_program(deltas):
    d0, d1, d2 = deltas
    nc = bacc.Bacc("TRN2", target_bir_lowering=False, debug=False, num_devices=NC)

    dfs = nc.dram_tensor("fs", [17, 128, 128], F32, kind="ExternalInput")
    dfsy = nc.dram_tensor("fsy", [17, 128, 128], F32, kind="ExternalInput")
    dtab = nc.dram_tensor("tab", [128, 512], F32, kind="ExternalInput").ap()
    dwm = nc.dram_tensor("wm", [8, 128, 128], BF16, kind="ExternalInput")
    dbz = nc.dram_tensor("bz", [128, NCOL], F32, kind="ExternalInput").ap()
    dyt = nc.dram_tensor("yt", [128, NCOL], F32, kind="ExternalInput").ap()
    dx128 = nc.dram_tensor("x128", [128, XL], F32, kind="ExternalInput").ap()
    dx16k = nc.dram_tensor("x16k", [128, XL], F32, kind="ExternalInput").ap()
    dxs0 = nc.dram_tensor("xs0", [128, XL], F32, kind="ExternalInput").ap()
    dc0x = nc.dram_tensor("c0x", [128, 384], F32, kind="ExternalInput").ap()
    dc0y = nc.dram_tensor("c0y", [128, 384], F32, kind="ExternalInput").ap()
    dc0z = nc.dram_tensor("c0z", [128, 381], F32, kind="ExternalInput").ap()

    hvx = nc.dram_tensor("vx", [XL * 16384 * 3], F32, kind="ExternalOutput")
    hvy = nc.dram_tensor("vy", [XL * 16256 * 3], F32, kind="ExternalOutput")
    hvz = nc.dram_tensor("vz", [XL * 16256 * 3], F32, kind="ExternalOutput")
    dfc = nc.dram_tensor("fc", [128, XL * NCOL], I32, kind="ExternalOutput").ap()
    dfm = nc.dram_tensor("fm", [128, XL * NCOL], U8, kind="ExternalOutput").ap()

    with tile.TileContext(nc) as tc:
        with (tc.tile_pool(name="consts", bufs=1) as cpool,
              tc.tile_pool(name="psum", bufs=2, space="PSUM") as ppool,
              tc.tile_pool(name="gather", bufs=2) as gpool,
              tc.tile_pool(name="tmp", bufs=2) as tpool,
              tc.tile_pool(name="verts", bufs=2) as vpool):

            # ---- static loads
            tabs = cpool.tile([128, 512], F32)
            nc.sync.dma_start(tabs[:], dtab[:])
            wt = cpool.tile([128, 8 * 128], BF16)
            nc.sync.dma_start(wt[:], bass.AP(dwm, 0, [[128, 128], [16384, 8], [1, 128]]))
            sl = cpool.tile([128, 17 * 128], F32)
            nc.sync.dma_start(sl[:], bass.AP(dfs, 0, [[128, 128], [16384, 17], [1, 128]]))
            sly = cpool.tile([128, 17 * 128], F32, name="sly")
            nc.sync.dma_start(sly[:],
                              bass.AP(dfsy, 0, [[128, 128], [16384, 17], [1, 128]]))
            bzt = cpool.tile([128, NCOL], F32)
            nc.sync.dma_start(bzt[:], dbz[:])
            ytt = cpool.tile([128, NCOL], F32)
            nc.sync.dma_start(ytt[:], dyt[:])
            x128t = cpool.tile([128, XL], F32)
            nc.sync.dma_start(x128t[:], dx128[:])
            x16kt = cpool.tile([128, XL], F32)
            nc.sync.dma_start(x16kt[:], dx16k[:])
            xs0t = cpool.tile([128, XL], F32)
            nc.sync.dma_start(xs0t[:], dxs0[:])
            c0xt = cpool.tile([128, 384], F32)
            nc.sync.dma_start(c0xt[:], dc0x[:])
            c0yt = cpool.tile([128, 384], F32)
            nc.sync.dma_start(c0yt[:], dc0y[:])
            c0zt = cpool.tile([128, 381], F32)
            nc.sync.dma_start(c0zt[:], dc0z[:])

            pb = cpool.tile([128, 17 * 128], BF16)
            nc.vector.tensor_scalar(pb[:], sl[:], 0.0, None, AL.is_gt)

            for x in range(XL):
                # ------------ case ids via PE shift-matmuls
                cps = ppool.tile([128, 127], F32, name="cpsum")
                for m in range(8):
                    dx, dy, dz = m & 1, (m >> 1) & 1, (m >> 2) & 1
                    nc.tensor.matmul(
                        cps[:],
                        wt[:, 128 * m:128 * (m + 1)],
                        pb[:, 128 * (x + dx) + dz:128 * (x + dx) + dz + 127],
                        start=(m == 0), stop=(m == 7))
                idxt = tpool.tile([128, 127], I16, name="idxt")
                nc.scalar.activation(idxt[:], cps[:], ACTF.Copy, bias=0.0, scale=1.0)

                # ------------ gather G + masks
                ga = gpool.tile([128, NCOL * 2], F32, name="ga")
                nc.gpsimd.ap_gather(ga[:], tabs[:], idxt[:],
                                    channels=128, num_elems=256, d=2, num_idxs=NCOL)
                ev = ga[:].rearrange("p (i two) -> p i two", two=2)[:, :, 0]

                # ------------ faces assembly
                t1 = tpool.tile([128, NCOL], F32, name="t1")
                nc.vector.tensor_scalar(t1[:], ev, float(Y_OFF),
                                        x128t[:, x:x + 1], AL.is_ge, AL.mult)
                t2 = tpool.tile([128, NCOL], F32, name="t2")
                nc.vector.scalar_tensor_tensor(t2[:], ev, float(Z_OFF), ytt[:],
                                               AL.is_ge, AL.mult)
                t3 = tpool.tile([128, NCOL], F32, name="t3")
                nc.vector.tensor_tensor(t3[:], t1[:], t2[:], AL.subtract)
                t4 = tpool.tile([128, NCOL], F32, name="t4")
                nc.vector.scalar_tensor_tensor(t4[:], ev, x16kt[:, x:x + 1], bzt[:],
                                               AL.add, AL.add)
                fo = tpool.tile([128, NCOL], I32, name="fo")
                nc.vector.tensor_tensor(fo[:], t3[:], t4[:], AL.add)

                nc.sync.dma_start(dfc[:, x * NCOL:(x + 1) * NCOL], fo[:])

                mv = ga[:].rearrange("p (i two) -> p i two", two=2)[:, :, 1]
                mu8 = tpool.tile([128, NCOL], U8, name="mu8")
                nc.scalar.activation(mu8[:], mv, ACTF.Copy, bias=0.0, scale=1.0)
                nc.sync.dma_start(dfm[:, x * NCOL:(x + 1) * NCOL], mu8[:])

                # ------------ verts
                def interp(s1ap, s2ap, w, name):
                    d = vpool.tile([128, w], F32, name=name + "_d")
                    nc.vector.tensor_tensor(d[:], s1ap, s2ap, AL.subtract)
                    z0 = vpool.tile([128, w], F32, name=name + "_z")
                    nc.vector.tensor_scalar(z0[:], d[:], 0.0, None, AL.is_equal)
                    nc.vector.tensor_tensor(d[:], d[:], z0[:], AL.add)
                    r = vpool.tile([128, w], F32, name=name + "_r")
                    nc.vector.reciprocal(r[:], d[:])
                    t = vpool.tile([128, w], F32, name=name + "_t")
                    nc.vector.tensor_tensor(t[:], s1ap, r[:], AL.mult)
                    return t

                # x-edges: s1 = sl[x], s2 = sl[x+1]
                t = interp(sl[:, 128 * x:128 * (x + 1)],
                           sl[:, 128 * (x + 1):128 * (x + 2)], 128, "vx")
                vo = vpool.tile([128, 384], F32, name="vox")
                nc.vector.tensor_copy(_c3(vo[:], 1), _c3(c0xt[:], 1))
                nc.vector.tensor_copy(_c3(vo[:], 2), _c3(c0xt[:], 2))
                nc.vector.tensor_scalar(_c3(vo[:], 0), t[:], d0,
                                        xs0t[:, x:x + 1], AL.mult, AL.add)
                nc.sync.dma_start(
                    bass.AP(hvx, x * 16384 * 3, [[384, 128], [1, 384]]), vo[:])

                # y-edges: s1 = sl[x], s2 = sly[x]; rows j < 127
                t = interp(sl[:, 128 * x:128 * (x + 1)],
                           sly[:, 128 * x:128 * (x + 1)], 128, "vy")
                vo = vpool.tile([128, 384], F32, name="voy")
                nc.vector.tensor_copy(_c3(vo[0:127, :], 2), _c3(c0yt[0:127, :], 2))
                nc.vector.tensor_scalar(_c3(vo[0:127, :], 0), t[0:127, :], 0.0,
                                        xs0t[0:127, x:x + 1], AL.mult, AL.add)
                nc.vector.scalar_tensor_tensor(_c3(vo[0:127, :], 1), t[0:127, :],
                                               d1, _c3(c0yt[0:127, :], 1),
                                               AL.mult, AL.add)
                nc.sync.dma_start(
                    bass.AP(hvy, x * 16256 * 3, [[384, 127], [1, 384]]),
                    vo[0:127, :])

                # z-edges: s1 = sl[x][:, :127], s2 shifted by one column
                t = interp(sl[:, 128 * x:128 * x + 127],
                           sl[:, 128 * x + 1:128 * (x + 1)], 127, "vz")
                vo = vpool.tile([128, 381], F32, name="voz")
                nc.vector.tensor_copy(_c3(vo[:], 1), _c3(c0zt[:], 1))
                nc.vector.tensor_scalar(_c3(vo[:], 0), t[:], 0.0,
                                        xs0t[:, x:x + 1], AL.mult, AL.add)
                nc.vector.scalar_tensor_tensor(_c3(vo[:], 2), t[:], d2,
                                               _c3(c0zt[:], 2), AL.mult, AL.add)
                nc.sync.dma_start(
                    bass.AP(hvz, x * 16256 * 3, [[381, 128], [1, 381]]), vo[:])

    nc.compile()
    return nc


# ---------------------------------------------------------------- driver

@functools.lru_cache(maxsize=2)
def _get_program(deltas):
    return build_program(deltas)


def kernel(field, min_point, size, cases, masks, _trace=False, _return_res=False):
    field = np.asarray(field, np.float32)
    min_point = np.asarray(min_point, np.float32)
    size = np.asarray(size, np.float32)
    tab = build_tables(np.asarray(cases), np.asarray(masks))
    wm = build_wmats()

    in_maps = []
    deltas = None
    for k in range(NC):
        x0 = XL * k
        cst = build_consts(min_point, size, x0)
        deltas = cst["deltas"]
        fs = np.zeros((17, 128, 128), np.float32)
        hi = min(x0 + 17, GRID)
        fs[:hi - x0] = field[x0:hi]
        fsy = np.full((17, 128, 128), 1.0, np.float32)
        fsy[:, :127] = fs[:, 1:]
        in_maps.append({
            "fs": fs, "fsy": fsy, "tab": tab, "wm": wm,
            "bz": cst["bz"], "yt": cst["yt"], "x128": cst["x128"],
            "x16k": cst["x16k"], "xs0": cst["xs0"],
            "c0x": cst["c0x"], "c0y": cst["c0y"], "c0z": cst["c0z"],
        })

    nc = _get_program(deltas)
    res = run_bass_kernel_spmd(nc, in_maps, core_ids=list(range(NC)),
                               trace=_trace)

    nxe_c = XL * 16384
    nye_c = XL * 16256
    nfc_c = XL * 16129
    verts = np.empty((3 * Y_OFF, 3), np.float32)
    faces = np.empty((127 * 127 * 127 * 5, 3), np.int32)
    fmask = np.empty((127 * 127 * 127 * 5,), np.uint8)
    for k in range(NC):
        r = res.results[k]
        nx = nxe_c if k < 7 else 15 * 16384
        verts[k * nxe_c:k * nxe_c + nx] = r["vx"].reshape(-1, 3)[:nx]
        verts[Y_OFF + k * nye_c:Y_OFF + (k + 1) * nye_c] = r["vy"].reshape(-1, 3)
        verts[Z_OFF + k * nye_c:Z_OFF + (k + 1) * nye_c] = r["vz"].reshape(-1, 3)
        nlay = XL if k < 7 else 15
        fcarr = r["fc"].reshape(8, 16, XL, 127, 16)[:, :15]   # [g,s,x,z,y_in]
        blk = np.transpose(fcarr, (2, 0, 4, 3, 1)).reshape(XL, 128, 127, 15)
        faces[k * nfc_c * 5:k * nfc_c * 5 + nlay * 16129 * 5] = \
            blk[:nlay, :127].reshape(-1, 3)
        fmarr = r["fm"].reshape(8, 16, XL, 127, 16)[:, :5]    # [g,r,x,z,y_in]
        mblk = np.transpose(fmarr, (2, 0, 4, 3, 1)).reshape(XL, 128, 127, 5)
        fmask[k * nfc_c * 5:k * nfc_c * 5 + nlay * 16129 * 5] = \
            mblk[:nlay, :127].reshape(-1)
    out = (verts, faces, fmask.astype(bool))
    if _return_res:
        return out, res
    return out
